# revision 1
# baseline (speedup 1.0000x reference)
"""Chamfer distance L2 kernel for Trainium2 (8 NeuronCores).

Problem: B=32, N=M=4096, C=3 point clouds.
    D[b,n,m] = ||xyz1[b,n] - xyz2[b,m]||^2
    out[b]   = mean_n min_m D + mean_m min_n D

Strategy (per core, data-parallel over batch: 4 batches/core):
  - Augmented matmul trick: with xt = [x0,x1,x2, -0.5*||x||^2, 1] (K=5)
    and yt = [y0,y1,y2, 1, -0.5*||y||^2], the PE matmul computes
    S[n,m] = xt.T @ yt = x.y - 0.5||x||^2 - 0.5||y||^2 = -D[n,m]/2.
    So min_m D = -2 * max_m S  (all reductions become max over S).
  - Precision: D_min ~ 1e-3 arises from cancellation of O(1) terms, so
    reduced-precision inputs (bf16/f32r/fp16) are fatal, and fp32
    matmuls run at 4 cycles/row on the PE.  Solution: exact fp16 hi/lo
    split-GEMM folded into the contraction dim.  With xh = fp16(x),
    xl = fp16(x - xh):  x.y = xh.yh + xh.yl + xl.yh + xl.yl  exactly
    (fp16 products are exact in the fp32 PSUM accumulator; residual
    ~2^-22).  All 4 terms pack into ONE K=20 fp16 matmul
    (X20 = [xt_h|xt_h|xt_l|xt_l], Y20 = [yt_h|yt_l|yt_h|yt_l]),
    and matmul cost is K-independent: full fp32-split precision at
    fp16 speed (1 cycle/row).
  - ACT (ScalarE) drains PSUM fp32 -> SBUF fp16 (1 elem/cycle; it is
    the only cheap PSUM consumer and sits closest to PSUM).
  - DVE row direction: tensor_scalar(max, accum_out=max) fused reduce
    -> per-n max into rowmax[:, i]  (single-src op: 4x perf mode).
  - DVE col direction: in-place tensor_tensor max accumulation into
    cacc [128, 4096] fp16 (2x perf mode).
  - cacc partition reduction: DMA-transpose 128x128 fp16 chunks, then
    tensor_scalar fused max-reduce per chunk -> colmax[:, k].
  - Final means: reduce_sum + ones-matmul partition contraction,
    scale by -2/4096, DMA out.
"""

import numpy as np

B_FULL = 32
N_CORES = 8
B_LOC = B_FULL // N_CORES  # 4
N = 4096
M = 4096
C = 3

I_TILES = N // 128  # 32 row tiles
JG = 2              # psum groups per row tile
JW = M // JG        # 2048 columns per group
J_PER_G = JW // 512  # 4 matmuls per group
K_AUG = 5
K20 = 4 * K_AUG

# Lower bound for max reductions; true S values are > -100, and this stays
# representable in fp16.
NEG_BIG = -60000.0


def _build_bass():
    import concourse.bacc as bacc
    import concourse.mybir as mybir
    import concourse.tile as tile

    f32 = mybir.dt.float32
    f16 = mybir.dt.float16
    AL = mybir.AluOpType

    nc = bacc.Bacc("TRN2", target_bir_lowering=False, debug=False)

    xyz1 = nc.dram_tensor("xyz1", [B_LOC, N, C], f32, kind="ExternalInput")
    xyz2 = nc.dram_tensor("xyz2", [B_LOC, M, C], f32, kind="ExternalInput")
    out = nc.dram_tensor("out", [1, B_LOC], f32, kind="ExternalOutput")

    NT = N // 128  # wide-tile columns per coordinate (= 32)

    with tile.TileContext(nc) as tc:
        with (
            tc.tile_pool(name="consts", bufs=1) as consts,
            tc.tile_pool(name="coords", bufs=4) as coords_pool,
            tc.tile_pool(name="wide", bufs=2) as wide_pool,
            tc.tile_pool(name="sq", bufs=2) as sq_pool,
            tc.tile_pool(name="scr", bufs=4) as scr_pool,
            tc.tile_pool(name="cacc", bufs=2) as cacc_pool,
            tc.tile_pool(name="rmax", bufs=2) as rmax_pool,
            tc.tile_pool(name="tp", bufs=16) as tp_pool,
            tc.tile_pool(name="fin", bufs=1) as fin_pool,
            tc.tile_pool(name="psum", bufs=2, space="PSUM") as psum_pool,
        ):
            ones16_w = consts.tile([128, NT], f16)
            nc.vector.memset(ones16_w, 1.0)
            zeros16_w = consts.tile([128, NT], f16)
            nc.vector.memset(zeros16_w, 0.0)
            ones128 = consts.tile([128, 1], f32)
            nc.vector.memset(ones128, 1.0)
            dummy = consts.tile([128, M], f16)
            # sums[:, b] = per-partition partial sums of rowmax for batch b;
            # sums[:, B_LOC+b] = same for colmax.
            sums = consts.tile([128, 2 * B_LOC], f32)

            xts, yts = [], []
            for b in range(B_LOC):
                # ---- build hi/lo split augmented matrices [20, npts] f16 ----
                # X20 blocks: [h, h, l, l];  Y20 blocks: [h, l, h, l]
                # so sum_k X20[k].Y20[k] = (xt_h+xt_l).(yt_h+yt_l) exactly.
                xt = coords_pool.tile([K20, N], f16, tag="xt", name=f"xt{b}")
                yt = coords_pool.tile([K20, M], f16, tag="yt", name=f"yt{b}")
                xts.append(xt)
                yts.append(yt)

                for (src, dst, npts, const_row, sq_row, xpat) in (
                    (xyz2, yt, M, 3, 4, False),
                    (xyz1, xt, N, 4, 3, True),
                ):
                    nt_cnt = npts // 128
                    # wide load [128, nt, 3] fp32 (point n = nt*128 + p)
                    w = wide_pool.tile([128, nt_cnt * C], f32, tag="w")
                    nc.sync.dma_start(
                        out=w,
                        in_=src[b].rearrange("(nt p) c -> p nt c", p=128),
                    )
                    # hi/lo split of coordinates (wide, cheap).  The hi copy
                    # also deinterleaves [nt,c] -> [c,nt] so each coordinate
                    # row becomes a contiguous [128, nt] slice (cheap DMA).
                    w_cm = w.rearrange("p (nt c) -> p c nt", c=C)
                    wh = wide_pool.tile([128, nt_cnt * C], f16, tag="wh")
                    nc.vector.tensor_copy(
                        wh.rearrange("p (c nt) -> p nt c", c=C),
                        w.rearrange("p (nt c) -> p nt c", c=C),
                    )
                    whup = wide_pool.tile([128, nt_cnt * C], f32, tag="whup")
                    nc.vector.tensor_copy(whup, wh)
                    wl = wide_pool.tile([128, nt_cnt * C], f16, tag="wl")
                    nc.vector.tensor_sub(wl, w_cm, whup)
                    # -0.5*||.||^2 and its hi/lo split
                    wsq = wide_pool.tile([128, nt_cnt * C], f32, tag="wsq")
                    nc.vector.tensor_mul(wsq, w, w)
                    sq = sq_pool.tile([128, nt_cnt], f32, tag="sq")
                    nc.vector.tensor_reduce(
                        out=sq,
                        in_=wsq.rearrange("p (nt c) -> p nt c", c=C),
                        axis=mybir.AxisListType.X,
                        op=AL.add,
                    )
                    nc.vector.tensor_scalar_mul(sq, sq, -0.5)
                    sqh = sq_pool.tile([128, nt_cnt], f16, tag="sqh")
                    nc.vector.tensor_copy(sqh, sq)
                    squp = sq_pool.tile([128, nt_cnt], f32, tag="squp")
                    nc.vector.tensor_copy(squp, sqh)
                    sql = sq_pool.tile([128, nt_cnt], f16, tag="sql")
                    nc.vector.tensor_sub(sql, sq, squp)

                    # assemble the 4 K-blocks via SBUF->SBUF gather DMAs.
                    # row element order is n = p*nt_cnt + nt (matches the
                    # wide layout, so the sq rows are contiguous writes).
                    # All row sources are DVE-written tiles (engine->DMA
                    # deps are reliably tracked, unlike DMA->DMA).
                    xblks = "hhll" if xpat else "hlhl"
                    for rep in range(4):
                        hi = xblks[rep] == "h"
                        base = rep * K_AUG
                        csrc = wh if hi else wl
                        for c in range(C):
                            nc.gpsimd.dma_start(
                                out=dst[base + c : base + c + 1, :],
                                in_=csrc[:, c * nt_cnt : (c + 1) * nt_cnt],
                            )
                        nc.sync.dma_start(
                            out=dst[base + sq_row : base + sq_row + 1, :],
                            in_=(sqh if hi else sql)[:, :],
                        )
                        nc.sync.dma_start(
                            out=dst[base + const_row : base + const_row + 1, :],
                            in_=(ones16_w if hi else zeros16_w)[:, :nt_cnt],
                        )

            for b in range(B_LOC):
                xt, yt = xts[b], yts[b]
                # ---- main tiles ----
                cacc = cacc_pool.tile([128, M], f16, tag="cacc")
                rowmax = rmax_pool.tile([128, I_TILES], f16, tag="rowmax")

                for i in range(I_TILES):
                    scr = scr_pool.tile([128, M], f16, tag="scr")
                    for jg in range(JG):
                        pt = psum_pool.tile([128, JW], f32, tag="ps")
                        for j2 in range(J_PER_G):
                            j = jg * J_PER_G + j2
                            nc.tensor.matmul(
                                pt[:, j2 * 512 : (j2 + 1) * 512],
                                lhsT=xt[:, i * 128 : (i + 1) * 128],
                                rhs=yt[:, j * 512 : (j + 1) * 512],
                                start=True,
                                stop=True,
                            )
                        nc.scalar.copy(scr[:, jg * JW : (jg + 1) * JW], pt[:])
                    # row direction: per-n max over all m, fused reduce.
                    # Single-src op -> eligible for DVE 4x perf mode.
                    nc.vector.tensor_scalar(
                        dummy[:],
                        scr[:],
                        NEG_BIG,
                        None,
                        AL.max,
                        AL.max,
                        accum_out=rowmax[:, i : i + 1],
                    )
                    # col direction: elementwise max accumulate over i
                    if i == 0:
                        nc.vector.tensor_copy(cacc[:], scr[:])
                    else:
                        nc.vector.tensor_tensor(cacc[:], cacc[:], scr[:], AL.max)

                # ---- per-batch reductions ----
                nc.vector.tensor_reduce(
                    out=sums[:, b : b + 1],
                    in_=rowmax[:],
                    axis=mybir.AxisListType.X,
                    op=AL.add,
                )
                colmax = rmax_pool.tile([128, M // 128], f16, tag="colmax")
                for k in range(M // 128):
                    tp = tp_pool.tile([128, 128], f16, tag="tp")
                    nc.sync.dma_start(
                        out=tp, in_=cacc[:, k * 128 : (k + 1) * 128], transpose=True
                    )
                    nc.vector.tensor_scalar(
                        dummy[:, 0:128],
                        tp[:],
                        NEG_BIG,
                        None,
                        AL.max,
                        AL.max,
                        accum_out=colmax[:, k : k + 1],
                    )
                nc.vector.tensor_reduce(
                    out=sums[:, B_LOC + b : B_LOC + b + 1],
                    in_=colmax[:],
                    axis=mybir.AxisListType.X,
                    op=AL.add,
                )

            # ---- final: contract partitions via ones-matmul ----
            ps_fin = psum_pool.tile([1, 2 * B_LOC], f32, tag="ps")
            nc.tensor.matmul(ps_fin, lhsT=ones128, rhs=sums, start=True, stop=True)
            tmp8 = fin_pool.tile([1, 2 * B_LOC], f32)
            nc.scalar.copy(tmp8, ps_fin)
            tmp4 = fin_pool.tile([1, B_LOC], f32)
            nc.vector.tensor_add(tmp4, tmp8[:, 0:B_LOC], tmp8[:, B_LOC : 2 * B_LOC])
            nc.vector.tensor_scalar_mul(tmp4, tmp4, -2.0 / 4096.0)
            nc.sync.dma_start(out=out[:, :], in_=tmp4)

    nc.compile()
    return nc


_NC_CACHE = {}


def _get_nc():
    if "nc" not in _NC_CACHE:
        _NC_CACHE["nc"] = _build_bass()
    return _NC_CACHE["nc"]


def kernel(xyz1: np.ndarray, xyz2: np.ndarray) -> np.ndarray:
    from concourse.bass_utils import run_bass_kernel_spmd

    nc = _get_nc()
    xyz1 = np.ascontiguousarray(np.asarray(xyz1, dtype=np.float32))
    xyz2 = np.ascontiguousarray(np.asarray(xyz2, dtype=np.float32))
    in_maps = [
        {
            "xyz1": xyz1[c * B_LOC : (c + 1) * B_LOC],
            "xyz2": xyz2[c * B_LOC : (c + 1) * B_LOC],
        }
        for c in range(N_CORES)
    ]
    res = run_bass_kernel_spmd(nc, in_maps, core_ids=list(range(N_CORES)))
    out = np.concatenate([r["out"].reshape(B_LOC) for r in res.results])
    return out.astype(np.float32)


if __name__ == "__main__":
    rng = np.random.default_rng(0)
    a = rng.standard_normal((B_FULL, N, C), dtype=np.float32)
    b = rng.standard_normal((B_FULL, M, C), dtype=np.float32)
    r = kernel(a, b)
    print(r)

